# revision 3
# baseline (speedup 1.0000x reference)
"""Trainium2 Bass kernel for nn_Critic (branch MLPs -> 255-step LSTM -> head).

Strategy (hardcoded, 8 cores, data-parallel over batch B=512 -> 64/core):
  - Everything feature-major on chip: vectors are [feature_chunk(128), batch(64)].
  - bf16 matmul inputs, fp32 PSUM/gates/cell state.
  - PSUM z[p, bank, col]: bank = t mod 8 holds step t's full gate pre-activation
    z_t^T [1024, 64] as 8 m-chunks of 64 cols (col = m*64). Gate order i,f,g,o
    so sigmoid(i,f) reads cols 0:256, tanh(g) 256:384, sigmoid(o) 384:512 --
    all contiguous 1-dim APs.
  - Each bank is (re)initialized with the gate bias bl via a K=8 indicator
    matmul (start=True: clears has_written so later matmuls accumulate), then
    zx = Wk^T x_t accumulates (start=False), then the recurrent Wrk^T h
    matmuls accumulate. Init+zx for steps t+5..t+8 is emitted every 4th step,
    so PE fill work trickles into the ACT/DVE tail of each step instead of
    bursting at round boundaries.
"""

import numpy as np
import ml_dtypes

import concourse.bass as bass
import concourse.mybir as mybir
import concourse.tile as tile
from concourse import bacc
from concourse.bass_utils import run_bass_kernel_spmd

BF16 = mybir.dt.bfloat16
F32 = mybir.dt.float32
AF = mybir.ActivationFunctionType

NC = 8          # cores
B = 512
BC = B // NC    # 64 batch per core
T = 255         # real steps
TP = 256        # padded steps
U = 256
DIN = 256


def build_nc():
    nc = bacc.Bacc(None, target_bir_lowering=False)

    d_mot = nc.dram_tensor("mot", [64, BC], BF16, kind="ExternalInput")
    d_rob = nc.dram_tensor("rob", [128, BC], BF16, kind="ExternalInput")
    d_re = nc.dram_tensor("re_", [128, BC], BF16, kind="ExternalInput")
    d_im = nc.dram_tensor("im_", [128, BC], BF16, kind="ExternalInput")
    d_seq = nc.dram_tensor("seq", [2, 128, TP * BC], BF16, kind="ExternalInput")
    d_wm = nc.dram_tensor("wm", [64, 256], BF16, kind="ExternalInput")
    d_wr = nc.dram_tensor("wr", [128, 256], BF16, kind="ExternalInput")
    d_wre = nc.dram_tensor("wre", [128, 128], BF16, kind="ExternalInput")
    d_wim = nc.dram_tensor("wim", [128, 128], BF16, kind="ExternalInput")
    d_wc = nc.dram_tensor("wc", [128, 6, 256], BF16, kind="ExternalInput")
    d_wk = nc.dram_tensor("wk", [128, 2, 1024], BF16, kind="ExternalInput")
    d_wrk = nc.dram_tensor("wrk", [128, 2, 1024], BF16, kind="ExternalInput")
    d_wo = nc.dram_tensor("wo", [128, 2, 1], BF16, kind="ExternalInput")
    d_bm = nc.dram_tensor("bm2", [128, 2], F32, kind="ExternalInput")
    d_br = nc.dram_tensor("br2", [128, 2], F32, kind="ExternalInput")
    d_bre = nc.dram_tensor("bre1", [128, 1], F32, kind="ExternalInput")
    d_bim = nc.dram_tensor("bim1", [128, 1], F32, kind="ExternalInput")
    d_bc = nc.dram_tensor("bc2", [128, 2], F32, kind="ExternalInput")
    d_bo = nc.dram_tensor("bo1", [1, 1], F32, kind="ExternalInput")
    d_blw = nc.dram_tensor("blw8", [8, 128], BF16, kind="ExternalInput")
    d_ind = nc.dram_tensor("ind8", [8, 512], BF16, kind="ExternalInput")
    d_y = nc.dram_tensor("y", [1, BC], F32, kind="ExternalOutput")

    with tile.TileContext(nc) as tc:
        with (
            tc.tile_pool(name="sb", bufs=1) as sb,
            tc.tile_pool(name="rot", bufs=3) as rot,
        ):
            t_wk = sb.tile([128, 2, 1024], BF16, tag="wk")
            t_wrk = sb.tile([128, 2, 1024], BF16, tag="wrk")
            t_blw = sb.tile([8, 128], BF16, tag="blw")
            t_ind = sb.tile([8, 512], BF16, tag="ind")
            t_seq0 = sb.tile([128, TP * BC], BF16, tag="seq0")
            t_seq1 = sb.tile([128, TP * BC], BF16, tag="seq1")
            t_wm = sb.tile([64, 256], BF16, tag="wm")
            t_wr = sb.tile([128, 256], BF16, tag="wr")
            t_wre = sb.tile([128, 128], BF16, tag="wre")
            t_wim = sb.tile([128, 128], BF16, tag="wim")
            t_wc = sb.tile([128, 6, 256], BF16, tag="wc")
            t_wo = sb.tile([128, 2, 1], BF16, tag="wo")
            t_mot = sb.tile([64, BC], BF16, tag="mot")
            t_rob = sb.tile([128, BC], BF16, tag="rob")
            t_re = sb.tile([128, BC], BF16, tag="re")
            t_im = sb.tile([128, BC], BF16, tag="im")
            t_bm = sb.tile([128, 2], F32, tag="bm")
            t_br = sb.tile([128, 2], F32, tag="br")
            t_bre = sb.tile([128, 1], F32, tag="bre")
            t_bim = sb.tile([128, 1], F32, tag="bim")
            t_bc = sb.tile([128, 2], F32, tag="bc")
            t_bo = sb.tile([1, 1], F32, tag="bo")
            t_h = sb.tile([128, 2 * BC], BF16, tag="h")   # h^T (chunk k at cols k*64)
            t_c = sb.tile([128, 2 * BC], F32, tag="c")    # c^T
            t_cat = sb.tile([128, 6, BC], BF16, tag="cat")
            t_y = sb.tile([1, BC], F32, tag="y")

            nc.sync.dma_start(t_wm[:], d_wm[:])
            nc.sync.dma_start(t_wr[:], d_wr[:])
            nc.sync.dma_start(t_wre[:], d_wre[:])
            nc.sync.dma_start(t_wim[:], d_wim[:])
            nc.sync.dma_start(t_wc[:], d_wc[:])
            nc.sync.dma_start(t_mot[:], d_mot[:])
            nc.sync.dma_start(t_rob[:], d_rob[:])
            nc.sync.dma_start(t_re[:], d_re[:])
            nc.sync.dma_start(t_im[:], d_im[:])
            nc.sync.dma_start(t_bm[:], d_bm[:])
            nc.sync.dma_start(t_br[:], d_br[:])
            nc.sync.dma_start(t_bre[:], d_bre[:])
            nc.sync.dma_start(t_bim[:], d_bim[:])
            nc.sync.dma_start(t_bc[:], d_bc[:])
            nc.sync.dma_start(t_bo[:], d_bo[:])
            nc.sync.dma_start(t_wk[:], d_wk[:])
            nc.sync.dma_start(t_wrk[:], d_wrk[:])
            nc.sync.dma_start(t_blw[:], d_blw[:])
            nc.sync.dma_start(t_ind[:], d_ind[:])
            nc.sync.dma_start(t_wo[:], d_wo[:])
            CH = 16 * BC
            for ch in range(TP // 16):
                nc.sync.dma_start(
                    t_seq0[:, ch * CH:(ch + 1) * CH], d_seq[0, :, ch * CH:(ch + 1) * CH])
                nc.sync.dma_start(
                    t_seq1[:, ch * CH:(ch + 1) * CH], d_seq[1, :, ch * CH:(ch + 1) * CH])
            t_seq = [t_seq0, t_seq1]

            # ---- front-end branch MLPs -> state -> h0, c0 ----
            with tc.tile_pool(name="fp", bufs=1, space="PSUM") as fp:
                p6 = fp.tile([128, 6, BC], F32, tag="p6")
                for m in range(2):
                    nc.tensor.matmul(p6[:, m, :], t_wm[:, m * 128:(m + 1) * 128],
                                     t_mot[:], start=True, stop=True)
                for m in range(2):
                    nc.tensor.matmul(p6[:, 2 + m, :], t_wr[:, m * 128:(m + 1) * 128],
                                     t_rob[:], start=True, stop=True)
                nc.tensor.matmul(p6[:, 4, :], t_wre[:], t_re[:], start=True, stop=True)
                nc.tensor.matmul(p6[:, 5, :], t_wim[:], t_im[:], start=True, stop=True)
                for m in range(2):
                    nc.scalar.activation(t_cat[:, m, :], p6[:, m, :], AF.Relu,
                                         bias=t_bm[:, m:m + 1])
                for m in range(2):
                    nc.scalar.activation(t_cat[:, 2 + m, :], p6[:, 2 + m, :], AF.Relu,
                                         bias=t_br[:, m:m + 1])
                nc.scalar.activation(t_cat[:, 4, :], p6[:, 4, :], AF.Relu,
                                     bias=t_bre[:, 0:1])
                nc.scalar.activation(t_cat[:, 5, :], p6[:, 5, :], AF.Relu,
                                     bias=t_bim[:, 0:1])
                pst = fp.tile([128, 2, BC], F32, tag="pst")
                for mo in range(2):
                    for kc in range(6):
                        nc.tensor.matmul(
                            pst[:, mo, :],
                            t_wc[:, kc, mo * 128:(mo + 1) * 128],
                            t_cat[:, kc, :],
                            start=(kc == 0), stop=(kc == 5))
                for mo in range(2):
                    nc.scalar.activation(t_h[:, mo * BC:(mo + 1) * BC], pst[:, mo, :],
                                         AF.Relu, bias=t_bc[:, mo:mo + 1])
                    nc.scalar.activation(t_c[:, mo * BC:(mo + 1) * BC], pst[:, mo, :],
                                         AF.Relu, bias=t_bc[:, mo:mo + 1])

            # ---- LSTM recurrence ----
            with tc.tile_pool(name="zp", bufs=1, space="PSUM") as zp:
                z = zp.tile([128, 8, 8 * BC], F32, tag="z")   # [p, bank, m*64+b]

                def emit_bias(tp):
                    bk = tp % 8
                    nc.tensor.matmul(z[:, bk, :], t_blw[:], t_ind[:],
                                     start=True, stop=False, skip_group_check=True)

                def emit_zx(tps):
                    for m in range(8):
                        for k in range(2):
                            for tp in tps:
                                bk = tp % 8
                                nc.tensor.matmul(
                                    z[:, bk, m * BC:(m + 1) * BC],
                                    t_wk[:, k, m * 128:(m + 1) * 128],
                                    t_seq[k][:, tp * BC:(tp + 1) * BC],
                                    start=False, stop=False,
                                    skip_group_check=True)

                def emit_step(t):
                    bk = t % 8
                    for m in range(8):
                        for k in range(2):
                            nc.tensor.matmul(
                                z[:, bk, m * BC:(m + 1) * BC],
                                t_wrk[:, k, m * 128:(m + 1) * 128],
                                t_h[:, k * BC:(k + 1) * BC],
                                start=False, stop=(m == 7 and k == 1),
                                skip_group_check=True)
                    gb = rot.tile([128, 512], F32, tag="gb")
                    tm1 = rot.tile([128, 128], F32, tag="tm1")
                    tm2 = rot.tile([128, 128], F32, tag="tm2")
                    tmc = rot.tile([128, 128], F32, tag="tmc")
                    nc.scalar.activation(gb[:, 0:256], z[:, bk, 0:256], AF.Sigmoid)
                    nc.scalar.activation(gb[:, 256:384], z[:, bk, 256:384], AF.Tanh)
                    # c1 = sig(f)*c ; t1 = sig(i)*tanh(g)
                    nc.vector.tensor_mul(tm2[:], gb[:, 128:256], t_c[:])
                    nc.vector.tensor_mul(tm1[:], gb[:, 0:128], gb[:, 256:384])
                    nc.scalar.activation(gb[:, 384:512], z[:, bk, 384:512], AF.Sigmoid)
                    nc.vector.tensor_add(t_c[:], tm2[:], tm1[:])
                    nc.scalar.activation(tmc[:], t_c[:], AF.Tanh)
                    nc.vector.tensor_mul(t_h[:], gb[:, 384:512], tmc[:])

                for tp in range(8):
                    emit_bias(tp)
                emit_zx([0, 1, 2, 3])
                emit_zx([4, 5, 6, 7])
                for t in range(T):
                    emit_step(t)
                    if t % 4 == 3:
                        tps = [x for x in range(t + 5, t + 9) if x < T]
                        if tps:
                            for tp in tps:
                                emit_bias(tp)
                            emit_zx(tps)

            # ---- output head ----
            with tc.tile_pool(name="hp", bufs=1, space="PSUM") as hp:
                py = hp.tile([1, BC], F32, tag="py")
                for k in range(2):
                    nc.tensor.matmul(py[:], t_wo[:, k, :], t_h[:, k * BC:(k + 1) * BC],
                                     start=(k == 0), stop=(k == 1))
                nc.scalar.activation(t_y[:], py[:], AF.Relu, bias=t_bo[:, 0:1])
            nc.sync.dma_start(d_y[:], t_y[:])

    nc.compile()
    return nc


_NC_CACHE = None


def _prep_inputs(inputs):
    """Shard + lay out the full-problem inputs into 8 per-core in_maps."""
    bf = ml_dtypes.bfloat16
    f32 = np.float32

    hist = np.asarray(inputs["history"], f32)     # [B, 128, 256]
    act = np.asarray(inputs["action"], f32)       # [B, 128, 256]
    seq = np.concatenate([hist[:, :127], act], axis=1)          # [B, 255, 256]
    seq = np.concatenate(
        [seq, np.zeros((B, 1, DIN), f32)], axis=1)              # [B, 256, 256]

    Wk = np.asarray(inputs["Wk"], f32)            # [256, 1024]
    Wrk = np.asarray(inputs["Wrk"], f32)
    bl = np.asarray(inputs["bl"], f32)            # [1024]
    wk_p = np.ascontiguousarray(
        Wk.reshape(2, 128, 1024).transpose(1, 0, 2)).astype(bf)   # [128,2,1024]
    wrk_p = np.ascontiguousarray(
        Wrk.reshape(2, 128, 1024).transpose(1, 0, 2)).astype(bf)
    blw8 = np.ascontiguousarray(bl.reshape(8, 128)).astype(bf)    # [8,128]
    ind8 = np.zeros((8, 512), f32)
    for j in range(8):
        ind8[j, j * 64:(j + 1) * 64] = 1.0
    ind8 = ind8.astype(bf)
    Wc = np.asarray(inputs["Wc"], f32)            # [768, 256]
    wc_p = np.ascontiguousarray(
        Wc.reshape(6, 128, 256).transpose(1, 0, 2)).astype(bf)    # [128,6,256]
    Wo = np.asarray(inputs["Wo"], f32)            # [256, 1]
    wo_p = np.ascontiguousarray(
        Wo.reshape(2, 128, 1).transpose(1, 0, 2)).astype(bf)      # [128,2,1]

    def bias2(v, chunks):
        return np.ascontiguousarray(np.asarray(v, f32).reshape(chunks, 128).T)

    shared = {
        "wm": np.asarray(inputs["Wm"], f32).astype(bf),
        "wr": np.asarray(inputs["Wr"], f32).astype(bf),
        "wre": np.asarray(inputs["Wre"], f32).astype(bf),
        "wim": np.asarray(inputs["Wim"], f32).astype(bf),
        "wc": wc_p, "wk": wk_p, "wrk": wrk_p, "wo": wo_p,
        "bm2": bias2(inputs["bm"], 2), "br2": bias2(inputs["br"], 2),
        "bre1": bias2(inputs["bre"], 1), "bim1": bias2(inputs["bim"], 1),
        "bc2": bias2(inputs["bc"], 2),
        "bo1": np.asarray(inputs["bo"], f32).reshape(1, 1),
        "blw8": blw8, "ind8": ind8,
    }

    mot = np.asarray(inputs["motion_state"], f32)
    rob = np.asarray(inputs["robot_state"], f32)
    real = np.concatenate([np.asarray(inputs["osc_state_real"], f32),
                           np.asarray(inputs["osc_real"], f32)], -1)
    imag = np.concatenate([np.asarray(inputs["osc_state_imag"], f32),
                           np.asarray(inputs["osc_imag"], f32)], -1)

    in_maps = []
    for c in range(NC):
        sl = slice(c * BC, (c + 1) * BC)
        # on-chip col = t*64 + b  (plain t-major)
        sc = seq[sl].reshape(BC, TP, 2, 128)           # [b, t, fk, fp]
        sc = np.ascontiguousarray(sc.transpose(2, 3, 1, 0)).astype(bf)
        m = dict(shared)
        m["seq"] = np.ascontiguousarray(sc.reshape(2, 128, TP * BC))
        m["mot"] = np.ascontiguousarray(mot[sl].T).astype(bf)
        m["rob"] = np.ascontiguousarray(rob[sl].T).astype(bf)
        m["re_"] = np.ascontiguousarray(real[sl].T).astype(bf)
        m["im_"] = np.ascontiguousarray(imag[sl].T).astype(bf)
        in_maps.append(m)
    return in_maps


def kernel(**inputs):
    global _NC_CACHE
    if _NC_CACHE is None:
        _NC_CACHE = build_nc()
    in_maps = _prep_inputs(inputs)
    res = run_bass_kernel_spmd(_NC_CACHE, in_maps, core_ids=list(range(NC)))
    out = np.concatenate(
        [np.asarray(res.results[c]["y"], np.float32).T for c in range(NC)], axis=0)
    return out  # [512, 1] float32


# revision 8
# speedup vs baseline: 1.1065x; 1.1065x over previous
"""Trainium2 Bass kernel for nn_Critic (branch MLPs -> 255-step LSTM -> head).

Strategy (hardcoded, 8 cores, data-parallel over batch B=512 -> 64/core):
  - Everything feature-major on chip: vectors are [feature_chunk(128), batch(64)].
  - bf16 matmul inputs, fp32 PSUM/gates/cell state.
  - PSUM z[p, bank, col]: bank = t mod 8 holds step t's full gate pre-activation
    z_t^T [1024, 64] as 8 m-chunks of 64 cols (col = m*64). Gate order i,f,g,o
    so sigmoid(i,f) reads cols 0:256, tanh(g) 256:384, sigmoid(o) 384:512 --
    all contiguous 1-dim APs.
  - Each bank is (re)initialized with the gate bias bl via a K=8 indicator
    matmul (start=True: clears has_written so later matmuls accumulate), then
    zx = Wk^T x_t accumulates (start=False), then the recurrent Wrk^T h
    matmuls accumulate. Init+zx for steps t+5..t+8 is emitted every 4th step,
    so PE fill work trickles into the ACT/DVE tail of each step instead of
    bursting at round boundaries.
"""

import numpy as np
import ml_dtypes

import concourse.bass as bass
import concourse.mybir as mybir
import concourse.tile as tile
from concourse import bacc
from concourse.bass_utils import run_bass_kernel_spmd

BF16 = mybir.dt.bfloat16
F32 = mybir.dt.float32
AF = mybir.ActivationFunctionType

NC = 8          # cores
B = 512
BC = B // NC    # 64 batch per core
T = 255         # real steps
TP = 256        # padded steps
U = 256
DIN = 256


def build_nc(use_bias_mm=True):
    nc = bacc.Bacc(None, target_bir_lowering=False)

    d_mot = nc.dram_tensor("mot", [64, BC], BF16, kind="ExternalInput")
    d_rob = nc.dram_tensor("rob", [128, BC], BF16, kind="ExternalInput")
    d_re = nc.dram_tensor("re_", [128, BC], BF16, kind="ExternalInput")
    d_im = nc.dram_tensor("im_", [128, BC], BF16, kind="ExternalInput")
    d_seq = nc.dram_tensor("seq", [2, 128, TP * BC], BF16, kind="ExternalInput")
    d_wm = nc.dram_tensor("wm", [64, 256], BF16, kind="ExternalInput")
    d_wr = nc.dram_tensor("wr", [128, 256], BF16, kind="ExternalInput")
    d_wre = nc.dram_tensor("wre", [128, 128], BF16, kind="ExternalInput")
    d_wim = nc.dram_tensor("wim", [128, 128], BF16, kind="ExternalInput")
    d_wc = nc.dram_tensor("wc", [128, 6, 256], BF16, kind="ExternalInput")
    d_wk = nc.dram_tensor("wk", [128, 2, 1024], BF16, kind="ExternalInput")
    d_wrk = nc.dram_tensor("wrk", [128, 2, 1024], BF16, kind="ExternalInput")
    d_wo = nc.dram_tensor("wo", [128, 2, 1], BF16, kind="ExternalInput")
    d_bm = nc.dram_tensor("bm2", [128, 2], F32, kind="ExternalInput")
    d_br = nc.dram_tensor("br2", [128, 2], F32, kind="ExternalInput")
    d_bre = nc.dram_tensor("bre1", [128, 1], F32, kind="ExternalInput")
    d_bim = nc.dram_tensor("bim1", [128, 1], F32, kind="ExternalInput")
    d_bc = nc.dram_tensor("bc2", [128, 2], F32, kind="ExternalInput")
    d_bo = nc.dram_tensor("bo1", [1, 1], F32, kind="ExternalInput")
    d_blw = nc.dram_tensor("blw8", [8, 128], BF16, kind="ExternalInput")
    d_ind = nc.dram_tensor("ind8", [8, 512], BF16, kind="ExternalInput")
    d_y = nc.dram_tensor("y", [1, BC], F32, kind="ExternalOutput")

    with tile.TileContext(nc) as tc:
        with (
            tc.tile_pool(name="sb", bufs=1) as sb,
            tc.tile_pool(name="rot", bufs=3) as rot,
        ):
            t_wk = sb.tile([128, 2, 1024], BF16, tag="wk")
            t_wrk = sb.tile([128, 2, 1024], BF16, tag="wrk")
            t_blw = sb.tile([8, 128], BF16, tag="blw")
            t_ind = sb.tile([8, 512], BF16, tag="ind")
            t_seq0 = sb.tile([128, TP * BC], BF16, tag="seq0")
            t_seq1 = sb.tile([128, TP * BC], BF16, tag="seq1")
            t_wm = sb.tile([64, 256], BF16, tag="wm")
            t_wr = sb.tile([128, 256], BF16, tag="wr")
            t_wre = sb.tile([128, 128], BF16, tag="wre")
            t_wim = sb.tile([128, 128], BF16, tag="wim")
            t_wc = sb.tile([128, 6, 256], BF16, tag="wc")
            t_wo = sb.tile([128, 2, 1], BF16, tag="wo")
            t_mot = sb.tile([64, BC], BF16, tag="mot")
            t_rob = sb.tile([128, BC], BF16, tag="rob")
            t_re = sb.tile([128, BC], BF16, tag="re")
            t_im = sb.tile([128, BC], BF16, tag="im")
            t_bm = sb.tile([128, 2], F32, tag="bm")
            t_br = sb.tile([128, 2], F32, tag="br")
            t_bre = sb.tile([128, 1], F32, tag="bre")
            t_bim = sb.tile([128, 1], F32, tag="bim")
            t_bc = sb.tile([128, 2], F32, tag="bc")
            t_bo = sb.tile([1, 1], F32, tag="bo")
            t_h = sb.tile([128, 2 * BC], BF16, tag="h")   # h^T (chunk k at cols k*64)
            t_c = sb.tile([128, 2 * BC], F32, tag="c")    # c^T
            t_cat = sb.tile([128, 6, BC], BF16, tag="cat")
            t_y = sb.tile([1, BC], F32, tag="y")

            nc.sync.dma_start(t_wm[:], d_wm[:])
            nc.sync.dma_start(t_wr[:], d_wr[:])
            nc.sync.dma_start(t_wre[:], d_wre[:])
            nc.sync.dma_start(t_wim[:], d_wim[:])
            nc.sync.dma_start(t_wc[:], d_wc[:])
            nc.sync.dma_start(t_mot[:], d_mot[:])
            nc.sync.dma_start(t_rob[:], d_rob[:])
            nc.sync.dma_start(t_re[:], d_re[:])
            nc.sync.dma_start(t_im[:], d_im[:])
            nc.sync.dma_start(t_bm[:], d_bm[:])
            nc.sync.dma_start(t_br[:], d_br[:])
            nc.sync.dma_start(t_bre[:], d_bre[:])
            nc.sync.dma_start(t_bim[:], d_bim[:])
            nc.sync.dma_start(t_bc[:], d_bc[:])
            nc.sync.dma_start(t_bo[:], d_bo[:])
            nc.sync.dma_start(t_wk[:], d_wk[:])
            nc.sync.dma_start(t_wrk[:], d_wrk[:])
            nc.sync.dma_start(t_blw[:], d_blw[:])
            nc.sync.dma_start(t_ind[:], d_ind[:])
            nc.sync.dma_start(t_wo[:], d_wo[:])
            CH = 16 * BC
            for ch in range(TP // 16):
                nc.sync.dma_start(
                    t_seq0[:, ch * CH:(ch + 1) * CH], d_seq[0, :, ch * CH:(ch + 1) * CH])
                nc.sync.dma_start(
                    t_seq1[:, ch * CH:(ch + 1) * CH], d_seq[1, :, ch * CH:(ch + 1) * CH])
            t_seq = [t_seq0, t_seq1]

            # ---- front-end branch MLPs -> state -> h0, c0 ----
            with tc.tile_pool(name="fp", bufs=1, space="PSUM") as fp:
                p6 = fp.tile([128, 6, BC], F32, tag="p6")
                for m in range(2):
                    nc.tensor.matmul(p6[:, m, :], t_wm[:, m * 128:(m + 1) * 128],
                                     t_mot[:], start=True, stop=True)
                for m in range(2):
                    nc.tensor.matmul(p6[:, 2 + m, :], t_wr[:, m * 128:(m + 1) * 128],
                                     t_rob[:], start=True, stop=True)
                nc.tensor.matmul(p6[:, 4, :], t_wre[:], t_re[:], start=True, stop=True)
                nc.tensor.matmul(p6[:, 5, :], t_wim[:], t_im[:], start=True, stop=True)
                for m in range(2):
                    nc.scalar.activation(t_cat[:, m, :], p6[:, m, :], AF.Relu,
                                         bias=t_bm[:, m:m + 1])
                for m in range(2):
                    nc.scalar.activation(t_cat[:, 2 + m, :], p6[:, 2 + m, :], AF.Relu,
                                         bias=t_br[:, m:m + 1])
                nc.scalar.activation(t_cat[:, 4, :], p6[:, 4, :], AF.Relu,
                                     bias=t_bre[:, 0:1])
                nc.scalar.activation(t_cat[:, 5, :], p6[:, 5, :], AF.Relu,
                                     bias=t_bim[:, 0:1])
                pst = fp.tile([128, 2, BC], F32, tag="pst")
                for mo in range(2):
                    for kc in range(6):
                        nc.tensor.matmul(
                            pst[:, mo, :],
                            t_wc[:, kc, mo * 128:(mo + 1) * 128],
                            t_cat[:, kc, :],
                            start=(kc == 0), stop=(kc == 5))
                for mo in range(2):
                    nc.scalar.activation(t_h[:, mo * BC:(mo + 1) * BC], pst[:, mo, :],
                                         AF.Relu, bias=t_bc[:, mo:mo + 1])
                    nc.scalar.activation(t_c[:, mo * BC:(mo + 1) * BC], pst[:, mo, :],
                                         AF.Relu, bias=t_bc[:, mo:mo + 1])

            # ---- LSTM recurrence ----
            with tc.tile_pool(name="zp", bufs=1, space="PSUM") as zp:
                z = zp.tile([128, 8, 8 * BC], F32, tag="z")   # [p, bank, m*64+b]

                def emit_bias(tp):
                    bk = tp % 8
                    nc.tensor.matmul(z[:, bk, :], t_blw[:], t_ind[:],
                                     start=True, stop=False, skip_group_check=True)

                def emit_zx(tps):
                    # When use_bias_mm is False the (m=0, k=0) matmul is the
                    # bank-clearing start=True of the accumulation round: it
                    # clears the whole bank's has_written, so the other k=0
                    # matmuls overwrite (bit clear) and k=1 accumulate. Safe
                    # because same-engine ready-order preserves emission order
                    # here (all k=0 deps are the same or later ACT reads).
                    for m in range(8):
                        for k in range(2):
                            for tp in tps:
                                bk = tp % 8
                                nc.tensor.matmul(
                                    z[:, bk, m * BC:(m + 1) * BC],
                                    t_wk[:, k, m * 128:(m + 1) * 128],
                                    t_seq[k][:, tp * BC:(tp + 1) * BC],
                                    start=(not use_bias_mm and m == 0 and k == 0),
                                    stop=False,
                                    skip_group_check=True)

                def emit_step(t):
                    bk = t % 8
                    for m in range(8):
                        for k in range(2):
                            nc.tensor.matmul(
                                z[:, bk, m * BC:(m + 1) * BC],
                                t_wrk[:, k, m * 128:(m + 1) * 128],
                                t_h[:, k * BC:(k + 1) * BC],
                                start=False, stop=(m == 7 and k == 1),
                                skip_group_check=True)
                    gb = rot.tile([128, 512], F32, tag="gb")
                    tm1 = rot.tile([128, 128], F32, tag="tm1")
                    tm2 = rot.tile([128, 128], F32, tag="tm2")
                    tmc = rot.tile([128, 128], F32, tag="tmc")
                    nc.scalar.activation(gb[:, 0:256], z[:, bk, 0:256], AF.Sigmoid)
                    nc.scalar.activation(gb[:, 256:384], z[:, bk, 256:384], AF.Tanh)
                    # c1 = sig(f)*c ; t1 = sig(i)*tanh(g)
                    nc.vector.tensor_mul(tm2[:], gb[:, 128:256], t_c[:])
                    nc.vector.tensor_mul(tm1[:], gb[:, 0:128], gb[:, 256:384])
                    nc.scalar.activation(gb[:, 384:512], z[:, bk, 384:512], AF.Sigmoid)
                    nc.vector.tensor_add(t_c[:], tm2[:], tm1[:])
                    nc.scalar.activation(tmc[:], t_c[:], AF.Tanh)
                    nc.vector.tensor_mul(t_h[:], gb[:, 384:512], tmc[:])

                if use_bias_mm:
                    for tp in range(8):
                        emit_bias(tp)
                emit_zx([0, 1, 2, 3])
                emit_zx([4, 5, 6, 7])
                for t in range(T):
                    emit_step(t)
                    if t % 4 == 3:
                        # init steps t+2..t+5: their banks' gates were read
                        # 3-6 steps ago, so the in-order PE queue never stalls
                        tps = [x for x in range(t + 2, t + 6) if 8 <= x < T]
                        if tps:
                            if use_bias_mm:
                                for tp in tps:
                                    emit_bias(tp)
                            emit_zx(tps)

            # ---- output head ----
            with tc.tile_pool(name="hp", bufs=1, space="PSUM") as hp:
                py = hp.tile([1, BC], F32, tag="py")
                for k in range(2):
                    nc.tensor.matmul(py[:], t_wo[:, k, :], t_h[:, k * BC:(k + 1) * BC],
                                     start=(k == 0), stop=(k == 1))
                nc.scalar.activation(t_y[:], py[:], AF.Relu, bias=t_bo[:, 0:1])
            nc.sync.dma_start(d_y[:], t_y[:])

    nc.compile()
    return nc


_NC_CACHE = None


def _prep_inputs(inputs):
    """Shard + lay out the full-problem inputs into 8 per-core in_maps."""
    bf = ml_dtypes.bfloat16
    f32 = np.float32

    hist = np.asarray(inputs["history"], f32)     # [B, 128, 256]
    act = np.asarray(inputs["action"], f32)       # [B, 128, 256]
    seq = np.concatenate([hist[:, :127], act], axis=1)          # [B, 255, 256]
    seq = np.concatenate(
        [seq, np.zeros((B, 1, DIN), f32)], axis=1)              # [B, 256, 256]

    Wk = np.asarray(inputs["Wk"], f32)            # [256, 1024]
    Wrk = np.asarray(inputs["Wrk"], f32)
    bl = np.asarray(inputs["bl"], f32)            # [1024]
    wk_p = np.ascontiguousarray(
        Wk.reshape(2, 128, 1024).transpose(1, 0, 2)).astype(bf)   # [128,2,1024]
    wrk_p = np.ascontiguousarray(
        Wrk.reshape(2, 128, 1024).transpose(1, 0, 2)).astype(bf)
    blw8 = np.ascontiguousarray(bl.reshape(8, 128)).astype(bf)    # [8,128]
    ind8 = np.zeros((8, 512), f32)
    for j in range(8):
        ind8[j, j * 64:(j + 1) * 64] = 1.0
    ind8 = ind8.astype(bf)
    Wc = np.asarray(inputs["Wc"], f32)            # [768, 256]
    wc_p = np.ascontiguousarray(
        Wc.reshape(6, 128, 256).transpose(1, 0, 2)).astype(bf)    # [128,6,256]
    Wo = np.asarray(inputs["Wo"], f32)            # [256, 1]
    wo_p = np.ascontiguousarray(
        Wo.reshape(2, 128, 1).transpose(1, 0, 2)).astype(bf)      # [128,2,1]

    def bias2(v, chunks):
        return np.ascontiguousarray(np.asarray(v, f32).reshape(chunks, 128).T)

    shared = {
        "wm": np.asarray(inputs["Wm"], f32).astype(bf),
        "wr": np.asarray(inputs["Wr"], f32).astype(bf),
        "wre": np.asarray(inputs["Wre"], f32).astype(bf),
        "wim": np.asarray(inputs["Wim"], f32).astype(bf),
        "wc": wc_p, "wk": wk_p, "wrk": wrk_p, "wo": wo_p,
        "bm2": bias2(inputs["bm"], 2), "br2": bias2(inputs["br"], 2),
        "bre1": bias2(inputs["bre"], 1), "bim1": bias2(inputs["bim"], 1),
        "bc2": bias2(inputs["bc"], 2),
        "bo1": np.asarray(inputs["bo"], f32).reshape(1, 1),
        "blw8": blw8, "ind8": ind8,
    }

    mot = np.asarray(inputs["motion_state"], f32)
    rob = np.asarray(inputs["robot_state"], f32)
    real = np.concatenate([np.asarray(inputs["osc_state_real"], f32),
                           np.asarray(inputs["osc_real"], f32)], -1)
    imag = np.concatenate([np.asarray(inputs["osc_state_imag"], f32),
                           np.asarray(inputs["osc_imag"], f32)], -1)

    in_maps = []
    for c in range(NC):
        sl = slice(c * BC, (c + 1) * BC)
        # on-chip col = t*64 + b  (plain t-major)
        sc = seq[sl].reshape(BC, TP, 2, 128)           # [b, t, fk, fp]
        sc = np.ascontiguousarray(sc.transpose(2, 3, 1, 0)).astype(bf)
        m = dict(shared)
        m["seq"] = np.ascontiguousarray(sc.reshape(2, 128, TP * BC))
        m["mot"] = np.ascontiguousarray(mot[sl].T).astype(bf)
        m["rob"] = np.ascontiguousarray(rob[sl].T).astype(bf)
        m["re_"] = np.ascontiguousarray(real[sl].T).astype(bf)
        m["im_"] = np.ascontiguousarray(imag[sl].T).astype(bf)
        in_maps.append(m)
    return in_maps


def kernel(**inputs):
    global _NC_CACHE
    use_bias_mm = bool(np.any(np.asarray(inputs["bl"])))
    if _NC_CACHE is None or _NC_CACHE[1] != use_bias_mm:
        _NC_CACHE = (build_nc(use_bias_mm), use_bias_mm)
    in_maps = _prep_inputs(inputs)
    res = run_bass_kernel_spmd(_NC_CACHE[0], in_maps, core_ids=list(range(NC)))
    out = np.concatenate(
        [np.asarray(res.results[c]["y"], np.float32).T for c in range(NC)], axis=0)
    return out  # [512, 1] float32
